# revision 27
# baseline (speedup 1.0000x reference)
"""Trainium2 Bass kernel for a 3-layer GAT block (DeepGATBlockV2).

Strategy (8-core SPMD, nodes partitioned by dst range):
  - ALL per-core constants + inputs are packed into ONE int16 blob input
    (per-call argument overhead through the PJRT tunnel is ~0.7 ms per
    array; 20 arrays -> 1).
  - Per layer, each core builds packed table rows for its 2500-node shard:
    bf16 [x(128) | a_s,a_d as 8 f32 | pad] = 256 bf16 = 512 B/row.
    AllGather -> full [N,256] bf16 table in DRAM (10.2 MB).
  - Edges (incl. self loops) are sorted by dst on the host, bucketed into
    per-core dst-blocks of 125 nodes, padded to a uniform C chunks of 128
    edge slots (pad slots: src=0, dstloc=-1 -> zero one-hot column).
  - Per block: dma_gather by src (512B: x+scores) and by dst (256B:
    scores), spread across 4 SWDGE queues; q = a_s[src]+a_d[dst],
    w = exp(leaky_relu(q)) on ACT; one-hot s0[e,c,n] and weighted
    sh[e,c,h,n] = s0*w built by TWO broadcast-AP DVE ops per block
    (bf16); PE accumulates s_hT[f,(h,n)] += Xg.T @ sh and
    den[(h,n)] += ones.T @ sh over chunks (bf16 matmuls, 1 cycle/row).
  - Block post: y = s_hT * bcast(1/den) (0.25 head-mean folded into
    W_gat), attT = sum_h W_h.T @ y_h + bias; residual + RMSNorm + FFN
    (bf16 matmuls) + RMSNorm, all feature-major.
  - RMSNorm rsqrt = exp(-0.5*ln(ms+eps)); Exp/Ln/Lrelu/Relu/Copy all
    live in one activation table set (forced via get_activation_tables
    patch) so the ACT engine never reloads its function table.
"""

import functools

import numpy as np

import concourse.bass as bass
import concourse.bacc as bacc
import concourse.hw_specs as hw_specs
import concourse.tile as tile
from concourse import mybir
from concourse.bass_utils import run_bass_kernel_spmd

F32 = mybir.dt.float32
BF16 = mybir.dt.bfloat16
I16 = mybir.dt.int16
AOT = mybir.AluOpType
ACT = mybir.ActivationFunctionType

EPS = 1.1920929e-07
NEG_SLOPE = 0.2
ABLATE = set()  # timing-ablation flags

# ---- activation-table forcing ----------------------------------------
# Exp's canonical table set lacks Ln and vice versa; both live in
# "natural_log_exp_and_others". Restrict Exp/Ln to that set so the
# table-load pass emits one load instead of alternating per block.
_COMBINED_SET = "natural_log_exp_and_others"
_orig_get_tables = hw_specs.get_activation_tables


@functools.cache
def _forced_tables(arch):
    t = {k: set(v) for k, v in _orig_get_tables(arch).items()}
    used = {ACT.Exp, ACT.Ln, ACT.Prelu, ACT.Relu, ACT.Copy, ACT.Identity,
            ACT.MemsetZero}
    if _COMBINED_SET in t and used <= t[_COMBINED_SET]:
        for name, funcs in t.items():
            if name != _COMBINED_SET:
                funcs -= used
    return t


hw_specs.get_activation_tables = _forced_tables
bacc.get_activation_tables = _forced_tables


def _np_bf16():
    import ml_dtypes
    return ml_dtypes.bfloat16


class _Blob:
    """Packs 2-D arrays into one flat int16 buffer, 256B-aligned rows."""

    def __init__(self):
        self.parts = []
        self.off = 0  # int16 elems
        self.secs = {}

    def add(self, name, arr):
        a = np.ascontiguousarray(arr)
        b = a.view(np.int16)
        rows, cols = b.shape
        pad = (-self.off) % 128
        if pad:
            self.parts.append(np.zeros(pad, np.int16))
            self.off += pad
        self.secs[name] = (self.off, rows, cols)
        self.parts.append(b.reshape(-1))
        self.off += rows * cols

    def finish(self):
        pad = (-self.off) % 128
        if pad:
            self.parts.append(np.zeros(pad, np.int16))
            self.off += pad
        return np.concatenate(self.parts)


def host_prep(inputs, cfg):
    """Returns (in_maps, C) -- per-core single-blob inputs."""
    N, E, CORES = cfg["N"], cfg["E"], cfg["CORES"]
    SHARD, BLK, BLOCKS = cfg["SHARD"], cfg["BLK"], cfg["BLOCKS"]
    L, D, H = cfg["L"], cfg["D"], cfg["H"]
    bf16 = _np_bf16()

    x = np.ascontiguousarray(np.asarray(inputs["x"], np.float32))
    ei = np.asarray(inputs["edge_index"], np.int64)
    src = ei[0]
    dst = ei[1]
    loops = np.arange(N, dtype=np.int64)
    src = np.concatenate([src, loops])
    dst = np.concatenate([dst, loops])
    order = np.argsort(dst, kind="stable")
    src, dst = src[order], dst[order]

    nblk_total = N // BLK
    blk_of = dst // BLK
    counts = np.bincount(blk_of, minlength=nblk_total)
    C = int(np.ceil(counts.max() / 128))
    cfg["C"] = C
    slots = C * 128

    srcs = np.zeros((CORES, BLOCKS, slots), np.int64)
    dloc = np.full((CORES, BLOCKS, slots), -1.0, np.float32)
    dsts = np.zeros((CORES, BLOCKS, slots), np.int64)
    starts = np.concatenate([[0], np.cumsum(counts)])
    for b in range(nblk_total):
        core, blk = b // BLOCKS, b % BLOCKS
        s, e = int(starts[b]), int(starts[b + 1])
        n = e - s
        srcs[core, blk, :n] = src[s:e]
        dsts[core, blk, :n] = dst[s:e]
        dsts[core, blk, n:] = b * BLK  # valid row for pad reads
        dloc[core, blk, :n] = (dst[s:e] - b * BLK).astype(np.float32)

    def wrap_idx(a):
        # a: [BLOCKS, ns] int -> int16 [128, BLOCKS * ns//16], slot j of
        # block b at [j % 16 (replicated x8), b*ns//16 + j//16]
        ns = a.shape[1]
        a16 = a.reshape(BLOCKS, ns // 16, 16).transpose(0, 2, 1)
        a16 = a16.reshape(1, BLOCKS * 16, ns // 16)
        cols = np.concatenate(
            [a16[0, b * 16:(b + 1) * 16, :] for b in range(BLOCKS)],
            axis=1)  # [16, BLOCKS*ns//16]
        assert a.max() < 2 ** 15
        return np.tile(cols.astype(np.int16), (8, 1))

    # dloc layout [128, BLOCKS*C]: [p, b*C + ch] = slot ch*128+p of block b
    dloc_t = dloc.reshape(CORES, BLOCKS, C, 128).transpose(0, 3, 1, 2) \
                 .reshape(CORES, 128, BLOCKS * C)

    Wg = np.asarray(inputs["W_gat"], np.float32)     # [L, D, H*D]
    a_s = np.asarray(inputs["att_src"], np.float32)  # [L, H, D]
    a_d = np.asarray(inputs["att_dst"], np.float32)
    wasd = np.zeros((D, L * 2 * H), np.float32)
    for l in range(L):
        for h in range(H):
            Wh = Wg[l][:, h * D:(h + 1) * D]
            wasd[:, l * 2 * H + h] = Wh @ a_s[l, h]
            wasd[:, l * 2 * H + H + h] = Wh @ a_d[l, h]

    def col3(name):  # [L, D] -> [D, L]
        return np.ascontiguousarray(np.asarray(inputs[name], np.float32).T)

    blob = _Blob()
    # 0.25 head-mean folded into W_gat; [D, L*H*D] d-major
    blob.add("wgat", (0.25 * Wg).transpose(1, 0, 2).reshape(D, L * H * D)
             .astype(bf16))
    blob.add("w1", np.asarray(inputs["W1"], np.float32)
             .transpose(1, 0, 2).reshape(D, L * D).astype(bf16))
    blob.add("w2", np.asarray(inputs["W2"], np.float32)
             .transpose(1, 0, 2).reshape(D, L * D).astype(bf16))
    blob.add("wasd", wasd)
    blob.add("bg", col3("bias_gat"))
    blob.add("b1", col3("b1"))
    blob.add("b2", col3("b2"))
    blob.add("n1", np.asarray(inputs["norm1_w"], np.float32).reshape(1, -1))
    blob.add("n2", np.asarray(inputs["norm2_w"], np.float32).reshape(1, -1))
    blob.add("iota", np.tile(np.arange(BLK, dtype=np.float32), (128, 1))
             .astype(bf16))
    blob.add("ident", np.eye(128, dtype=np.float32))
    blob.add("onescol", np.ones((128, 2), bf16))   # col 0 used
    blob.add("onesrow", np.ones((1, 128), bf16))
    blob.add("onesf", np.ones((128, 2), np.float32))  # col 0 used
    # head-selector for denominator broadcast: hsel[k, h*128+m] = (k==h)
    blob.add("hsel", np.eye(H, dtype=np.float32).repeat(128, axis=1)
             .astype(bf16))
    common_len = blob.off
    common_parts = list(blob.parts)
    common_secs = dict(blob.secs)

    in_maps = []
    blob_len = None
    for c in range(CORES):
        bl = _Blob()
        bl.parts = list(common_parts)
        bl.off = common_len
        bl.secs = dict(common_secs)
        bl.add("idxa", wrap_idx(srcs[c]))
        bl.add("idxb", wrap_idx(dsts[c]))
        bl.add("dloc", dloc_t[c].astype(bf16))
        bl.add("xin", x[c * SHARD:(c + 1) * SHARD])
        buf = bl.finish()
        blob_len = len(buf)
        cfg["SECS"] = bl.secs
        in_maps.append({"blob": buf})
    cfg["BLOB_LEN"] = blob_len
    return in_maps, C


def build_program(cfg, debug=False):
    N, CORES = cfg["N"], cfg["CORES"]
    SHARD, BLK, BLOCKS, C = cfg["SHARD"], cfg["BLK"], cfg["BLOCKS"], cfg["C"]
    L, D, H = cfg["L"], cfg["D"], cfg["H"]
    SECS = cfg["SECS"]
    TW = 256          # packed table row width (bf16 elems) = 512 B
    slots = C * 128
    NQ = cfg.get("NQ", 4)

    nc = bacc.Bacc("TRN2", target_bir_lowering=False, debug=debug,
                   num_devices=CORES, num_swdge_queues=NQ,
                   dynamic_dma_scratch_size=16384)

    blob = nc.dram_tensor("blob", [cfg["BLOB_LEN"]], I16,
                          kind="ExternalInput").ap()
    out = nc.dram_tensor("out", [SHARD, D], F32, kind="ExternalOutput").ap()

    def sec(name, dt=F32):
        off, rows, cols = SECS[name]
        v = blob[off:off + rows * cols].rearrange("(r c) -> r c", c=cols)
        return v if dt == I16 else v.bitcast(dt)

    with tile.TileContext(nc) as tc:
        with tc.tile_pool(name="persist", bufs=1) as pp, \
             tc.tile_pool(name="dram", bufs=1, space="DRAM") as dp, \
             tc.tile_pool(name="gath", bufs=3) as gp, \
             tc.tile_pool(name="sc", bufs=3) as scp, \
             tc.tile_pool(name="chunk", bufs=2) as cp, \
             tc.tile_pool(name="post", bufs=2) as pop, \
             tc.tile_pool(name="psA", bufs=1, space="PSUM") as psA, \
             tc.tile_pool(name="psB", bufs=2, space="PSUM") as psB:

            # ---- persistent SBUF ----
            idxa_s = pp.tile([128, BLOCKS * slots // 16], I16)
            idxb_s = pp.tile([128, BLOCKS * slots // 16], I16)
            dloc_s = pp.tile([128, BLOCKS * C], BF16)
            wasd_s = pp.tile([128, L * 2 * H], F32)
            wgat_s = pp.tile([128, L * H * D], BF16)
            w1_s = pp.tile([128, L * D], BF16)
            w2_s = pp.tile([128, L * D], BF16)
            bg_s = pp.tile([128, L], F32)
            b1_s = pp.tile([128, L], F32)
            b2_s = pp.tile([128, L], F32)
            n1_s = pp.tile([1, L * D], F32)
            n2_s = pp.tile([1, L * D], F32)
            iota_s = pp.tile([128, BLK], BF16)
            ident_s = pp.tile([128, 128], F32)
            onescol_s = pp.tile([128, 2], BF16)
            onesrow_s = pp.tile([1, 128], BF16)
            onesf_s = pp.tile([128, 2], F32)
            hsel_s = pp.tile([4, 4 * 128], BF16)
            xT = pp.tile([128, SHARD], F32)
            eps_s = pp.tile([1, 1], F32)

            dma = nc.sync.dma_start
            dma(idxa_s[:], sec("idxa", I16))
            dma(idxb_s[:], sec("idxb", I16))
            dma(dloc_s[:], sec("dloc", BF16))
            dma(wasd_s[:], sec("wasd"))
            dma(wgat_s[:], sec("wgat", BF16))
            dma(w1_s[:], sec("w1", BF16))
            dma(w2_s[:], sec("w2", BF16))
            dma(bg_s[:], sec("bg"))
            dma(b1_s[:], sec("b1"))
            dma(b2_s[:], sec("b2"))
            dma(n1_s[:], sec("n1"))
            dma(n2_s[:], sec("n2"))
            dma(iota_s[:], sec("iota", BF16))
            dma(ident_s[:], sec("ident"))
            dma(onescol_s[:], sec("onescol", BF16))
            dma(onesrow_s[:], sec("onesrow", BF16))
            dma(onesf_s[:], sec("onesf"))
            dma(hsel_s[:], sec("hsel", BF16))
            nc.vector.memset(eps_s[:], EPS)
            xin = sec("xin", F32)

            # ---- DRAM tables for gather + collective ----
            tshard = dp.tile([SHARD, TW], BF16)
            RP = cfg.get("REPS", 1)
            if CORES > 1:
                aspace = "Local" if "coll" in ABLATE else "Shared"
                tfulls = [dp.tile([N, TW], BF16, addr_space=aspace,
                                  tag=f"tfull{i}", name=f"tfull{i}")
                          for i in range(L * RP)]
            else:
                tfulls = [tshard] * (L * RP)

            # ---- init: transpose input shard to feature-major xT ----
            for b in range(BLOCKS):
                xr = gp.tile([BLK, D], F32, tag="xr")
                nc.sync.dma_start(xr[:], xin[b * BLK:(b + 1) * BLK, :])
                ps_t = psB.tile([D, BLK], F32, tag="pb")
                nc.tensor.transpose(ps_t[:], xr[:], ident_s[:BLK, :BLK])
                nc.scalar.copy(xT[:, b * BLK:(b + 1) * BLK], ps_t[:])

            def rmsnorm(z, nw_row, tag, zout=None):
                """z: SBUF [D, BLK] f32 -> z * rsqrt(mean(z^2)+eps) * w.
                rsqrt = exp(-0.5 * ln(ms + eps)): one act table set."""
                zsq = pop.tile([D, BLK], F32, tag=f"zsq{tag}")
                nc.vector.tensor_mul(zsq[:], z[:], z[:])
                ps_ss = psB.tile([1, BLK], F32, tag="pb_ss", bufs=1)
                nc.tensor.matmul(ps_ss[:], onesf_s[:, 0:1], zsq[:],
                                 start=True, stop=True)
                lnm = pop.tile([1, BLK], F32, tag=f"lnm{tag}")
                nc.scalar.activation(lnm[:], ps_ss[:], ACT.Ln,
                                     scale=1.0 / D, bias=eps_s[:])
                rin = pop.tile([1, BLK], F32, tag=f"rin{tag}")
                nc.scalar.activation(rin[:], lnm[:], ACT.Exp, scale=-0.5)
                ps_rb = psB.tile([D, BLK], F32, tag="pb")
                nc.tensor.matmul(ps_rb[:], nw_row, rin[:],
                                 start=True, stop=True)
                zn = zout if zout is not None else pop.tile(
                    [D, BLK], F32, tag=f"zn{tag}")
                nc.vector.tensor_mul(zn if zout is not None else zn[:],
                                     z[:], ps_rb[:])
                return zn

            for rep in range(cfg.get("REPS", 1)):
             for l in range(L):
                # ---- phase A: packed table rows [x bf16 | scores f32] ----
                for b in range(BLOCKS):
                    xb = xT[:, b * BLK:(b + 1) * BLK]
                    ps_a = psB.tile([BLK, 2 * H], F32, tag="pb")
                    nc.tensor.matmul(ps_a[:], xb,
                                     wasd_s[:, l * 2 * H:(l + 1) * 2 * H],
                                     start=True, stop=True)
                    ps_x = psB.tile([BLK, D], F32, tag="pb")
                    nc.tensor.transpose(ps_x[:], xb, ident_s[:])
                    tt = gp.tile([BLK, TW], BF16, tag="tt")
                    nc.scalar.copy(tt[:, 0:D], ps_x[:])
                    tt_f32 = tt[:].bitcast(F32)  # [BLK, TW//2]
                    nc.scalar.copy(tt_f32[:, 64:64 + 2 * H], ps_a[:])
                    nc.vector.memset(tt[:, D + 4 * H:TW], 0.0)
                    nc.sync.dma_start(tshard[b * BLK:(b + 1) * BLK, :], tt[:])

                # ---- phase B: AllGather ----
                if CORES > 1 and "coll" in ABLATE:
                    for s in range(CORES):
                        nc.sync.dma_start(
                            tfulls[rep * L + l][s * SHARD:(s + 1) * SHARD, :],
                            tshard[:])
                elif CORES > 1:
                    nc.gpsimd.collective_compute(
                        "AllGather", AOT.bypass,
                        replica_groups=[list(range(CORES))],
                        ins=[tshard.opt()], outs=[tfulls[rep * L + l].opt()])

                # ---- phase C/D: edge aggregation + block post ----
                def chunk_phase(b):
                    tf = tfulls[rep * L + l]
                    ga = gp.tile([128, C * TW], BF16, tag="ga")
                    gb = gp.tile([128, C * (TW // 2)], BF16, tag="gb")
                    ic0 = b * (slots // 16)
                    ic1 = (b + 1) * (slots // 16)
                    if "noga" in ABLATE:
                        pass
                    elif "ga" not in ABLATE:
                        nc.gpsimd.dma_gather(
                            ga[:].rearrange("p (c e) -> p c e", e=TW),
                            tf[:], idxa_s[:, ic0:ic1],
                            num_idxs=slots, num_idxs_reg=slots,
                            elem_size=TW, queue_num=(2 * b) % NQ,
                            single_packet=False)
                    else:
                        nc.sync.dma_start(
                            ga[:].rearrange("p (c e) -> p c e", e=TW),
                            tf[0:128 * C, :].rearrange(
                                "(c p) e -> p c e", p=128))
                    if "noga" in ABLATE:
                        pass
                    elif "ga" not in ABLATE:
                        nc.gpsimd.dma_gather(
                            gb[:].rearrange("p (c e) -> p c e", e=TW // 2),
                            tf[:, D:TW], idxb_s[:, ic0:ic1],
                            num_idxs=slots, num_idxs_reg=slots,
                            elem_size=TW // 2, elem_step=TW,
                            queue_num=(2 * b + 1) % NQ, single_packet=False)
                    else:
                        nc.sync.dma_start(
                            gb[:].rearrange("p (c e) -> p c e", e=TW // 2),
                            tf[0:128 * C, D:TW].rearrange(
                                "(c p) e -> p c e", p=128))
                    ga_f = ga[:].bitcast(F32).rearrange(
                        "p (c e) -> p c e", e=TW // 2)
                    gb_f = gb[:].bitcast(F32).rearrange(
                        "p (c e) -> p c e", e=TW // 4)

                    q = scp.tile([128, C * H], F32, tag="q")
                    lr = scp.tile([128, C * H], F32, tag="lr")
                    wex = scp.tile([128, C * H], BF16, tag="wex")
                    if "score" not in ABLATE:
                        nc.vector.tensor_add(
                            q[:].rearrange("p (c h) -> p c h", h=H),
                            ga_f[:, :, 64:64 + H], gb_f[:, :, H:2 * H])
                        nc.scalar.activation(lr[:], q[:], ACT.Prelu,
                                             alpha=NEG_SLOPE)
                        nc.scalar.activation(wex[:], lr[:], ACT.Exp)

                    s0 = cp.tile([128, C * BLK], BF16, tag="s0")
                    sh = cp.tile([128, C * H * BLK], BF16, tag="sh")
                    if "sdve" not in ABLATE:
                        nc.vector.tensor_tensor(
                            s0[:].rearrange("p (c n) -> p c n", n=BLK),
                            dloc_s[:, b * C:(b + 1) * C].unsqueeze(2)
                                .broadcast_to([128, C, BLK]),
                            iota_s[:].unsqueeze(1).broadcast_to(
                                [128, C, BLK]),
                            op=AOT.is_equal)
                        nc.vector.tensor_tensor(
                            sh[:].rearrange("p (c h n) -> p c h n",
                                            h=H, n=BLK),
                            s0[:].rearrange("p (c n) -> p c n", n=BLK)
                                .unsqueeze(2).broadcast_to([128, C, H, BLK]),
                            wex[:].rearrange("p (c h) -> p c h", h=H)
                                .unsqueeze(3).broadcast_to([128, C, H, BLK]),
                            op=AOT.mult)

                    ps_all = psA.tile([D, H * BLK], F32, tag="ps_all",
                                      name=f"ps_all_{l}_{b}", bufs=2)
                    ps_den = psA.tile([H, BLK], F32, tag="ps_den",
                                      name=f"ps_den_{l}_{b}", bufs=2)
                    gav = ga[:].rearrange("p (c e) -> p c e", e=TW)
                    for ch in range(C):
                        shc = sh[:, ch * H * BLK:(ch + 1) * H * BLK]
                        if "smm" not in ABLATE:
                            nc.tensor.matmul(ps_den[:],
                                             wex[:, ch * H:(ch + 1) * H],
                                             s0[:, ch * BLK:(ch + 1) * BLK],
                                             start=(ch == 0),
                                             stop=(ch == C - 1))
                            nc.tensor.matmul(ps_all[:], gav[:, ch, 0:D], shc,
                                             start=(ch == 0),
                                             stop=(ch == C - 1))
                    return ps_all, ps_den

                def post_phase(b, ps_all, ps_den):
                    if "post" in ABLATE:
                        return
                    xb = xT[:, b * BLK:(b + 1) * BLK]
                    rden = pop.tile([H, BLK], BF16, tag="rden")
                    with nc.allow_low_precision(reason="1/den fine in bf16"):
                        nc.vector.reciprocal(rden[:], ps_den[:])
                    ps_rb = psB.tile([128, H * BLK], F32, tag="pb")
                    for h in range(H):
                        nc.tensor.matmul(ps_rb[:, h * BLK:(h + 1) * BLK],
                                         hsel_s[:, h * 128:(h + 1) * 128],
                                         rden[:], start=True, stop=True)
                    rb = pop.tile([128, H * BLK], F32, tag="rb")
                    nc.scalar.copy(rb[:], ps_rb[:])
                    yh = pop.tile([128, H * BLK], BF16, tag="yh")
                    nc.vector.tensor_mul(yh[:], ps_all[:], rb[:])
                    ps_att = psB.tile([D, BLK], F32, tag="pb")
                    for h in range(H):
                        nc.tensor.matmul(
                            ps_att[:],
                            wgat_s[:, (l * H + h) * D:(l * H + h + 1) * D],
                            yh[:, h * BLK:(h + 1) * BLK],
                            start=(h == 0), stop=(h == H - 1))

                    z = pop.tile([D, BLK], F32, tag="z")
                    nc.vector.scalar_tensor_tensor(
                        z[:], ps_att[:], bg_s[:, l:l + 1], xb,
                        op0=AOT.add, op1=AOT.add)
                    zn1 = rmsnorm(z, n1_s[0:1, l * D:(l + 1) * D], "a")

                    zn1h = pop.tile([D, BLK], BF16, tag="zn1h")
                    nc.scalar.copy(zn1h[:], zn1[:])
                    ps_f1 = psB.tile([D, BLK], F32, tag="pb")
                    nc.tensor.matmul(ps_f1[:], w1_s[:, l * D:(l + 1) * D],
                                     zn1h[:], start=True, stop=True)
                    f1 = pop.tile([D, BLK], BF16, tag="f1")
                    nc.scalar.activation(f1[:], ps_f1[:], ACT.Relu,
                                         bias=b1_s[:, l:l + 1])
                    ps_f2 = psB.tile([D, BLK], F32, tag="pb")
                    nc.tensor.matmul(ps_f2[:], w2_s[:, l * D:(l + 1) * D],
                                     f1[:], start=True, stop=True)
                    z3 = pop.tile([D, BLK], F32, tag="z3")
                    nc.vector.scalar_tensor_tensor(
                        z3[:], ps_f2[:], b2_s[:, l:l + 1], zn1[:],
                        op0=AOT.add, op1=AOT.add)
                    rmsnorm(z3, n2_s[0:1, l * D:(l + 1) * D], "b", zout=xb)

                pending = None
                for b in range(BLOCKS):
                    handles = chunk_phase(b)
                    if pending is not None:
                        post_phase(pending[0], pending[1], pending[2])
                    pending = (b, *handles)
                post_phase(pending[0], pending[1], pending[2])

            # ---- output: transpose back to node-major ----
            for b in range(BLOCKS):
                ps_o = psB.tile([BLK, D], F32, tag="pb")
                nc.tensor.transpose(ps_o[:], xT[:, b * BLK:(b + 1) * BLK],
                                    ident_s[:])
                ot = gp.tile([BLK, D], F32, tag="ot")
                nc.scalar.copy(ot[:], ps_o[:])
                nc.sync.dma_start(out[b * BLK:(b + 1) * BLK, :], ot[:])

    nc.compile()
    return nc


FULL_CFG = dict(N=20000, E=320000, CORES=8, SHARD=2500, BLK=125, BLOCKS=20,
                C=None, L=3, D=128, H=4, NQ=4)


def kernel_run(inputs, trace=False):
    cfg = dict(FULL_CFG)
    in_maps, C = host_prep(inputs, cfg)
    nc = build_program(cfg)
    res = run_bass_kernel_spmd(nc, in_maps, list(range(cfg["CORES"])),
                               trace=trace)
    out = np.concatenate([r["out"] for r in res.results], axis=0)
    return out, res


def kernel(**inputs):
    out, _ = kernel_run(inputs)
    return out.astype(np.float32)


# revision 29
# speedup vs baseline: 1.2110x; 1.2110x over previous
"""Trainium2 Bass kernel for a 3-layer GAT block (DeepGATBlockV2).

Strategy (8-core SPMD, nodes partitioned by dst range):
  - ALL per-core constants + inputs are packed into ONE int16 blob input
    (per-call argument overhead through the PJRT tunnel is ~0.7 ms per
    array; 20 arrays -> 1).
  - Per layer, each core builds packed table rows for its 2500-node shard:
    bf16 [x(128) | a_s,a_d as 8 f32 | pad] = 256 bf16 = 512 B/row.
    AllGather -> full [N,256] bf16 table in DRAM (10.2 MB).
  - Edges (incl. self loops) are sorted by dst on the host, bucketed into
    per-core dst-blocks of 125 nodes, padded to a uniform C chunks of 128
    edge slots (pad slots: src=0, dstloc=-1 -> zero one-hot column).
  - Per block: dma_gather by src (512B: x+scores) and by dst (256B:
    scores), spread across 4 SWDGE queues; q = a_s[src]+a_d[dst],
    w = exp(leaky_relu(q)) on ACT; one-hot s0[e,c,n] and weighted
    sh[e,c,h,n] = s0*w built by TWO broadcast-AP DVE ops per block
    (bf16); PE accumulates s_hT[f,(h,n)] += Xg.T @ sh and
    den[(h,n)] += ones.T @ sh over chunks (bf16 matmuls, 1 cycle/row).
  - Block post: y = s_hT * bcast(1/den) (0.25 head-mean folded into
    W_gat), attT = sum_h W_h.T @ y_h + bias; residual + RMSNorm + FFN
    (bf16 matmuls) + RMSNorm, all feature-major.
  - RMSNorm rsqrt = exp(-0.5*ln(ms+eps)); Exp/Ln/Lrelu/Relu/Copy all
    live in one activation table set (forced via get_activation_tables
    patch) so the ACT engine never reloads its function table.
"""

import functools

import numpy as np

import concourse.bass as bass
import concourse.bacc as bacc
import concourse.hw_specs as hw_specs
import concourse.tile as tile
from concourse import mybir
from concourse.bass_utils import run_bass_kernel_spmd

F32 = mybir.dt.float32
BF16 = mybir.dt.bfloat16
I16 = mybir.dt.int16
AOT = mybir.AluOpType
ACT = mybir.ActivationFunctionType

EPS = 1.1920929e-07
NEG_SLOPE = 0.2
ABLATE = set()  # timing-ablation flags

# ---- activation-table forcing ----------------------------------------
# Exp's canonical table set lacks Ln and vice versa; both live in
# "natural_log_exp_and_others". Restrict Exp/Ln to that set so the
# table-load pass emits one load instead of alternating per block.
_COMBINED_SET = "natural_log_exp_and_others"
_orig_get_tables = hw_specs.get_activation_tables


@functools.cache
def _forced_tables(arch):
    t = {k: set(v) for k, v in _orig_get_tables(arch).items()}
    used = {ACT.Exp, ACT.Ln, ACT.Prelu, ACT.Relu, ACT.Copy, ACT.Identity,
            ACT.MemsetZero}
    if _COMBINED_SET in t and used <= t[_COMBINED_SET]:
        for name, funcs in t.items():
            if name != _COMBINED_SET:
                funcs -= used
    return t


hw_specs.get_activation_tables = _forced_tables
bacc.get_activation_tables = _forced_tables


def _np_bf16():
    import ml_dtypes
    return ml_dtypes.bfloat16


class _Blob:
    """Packs 2-D arrays into one flat int16 buffer, 256B-aligned rows."""

    def __init__(self):
        self.parts = []
        self.off = 0  # int16 elems
        self.secs = {}

    def add(self, name, arr):
        a = np.ascontiguousarray(arr)
        b = a.view(np.int16)
        rows, cols = b.shape
        pad = (-self.off) % 128
        if pad:
            self.parts.append(np.zeros(pad, np.int16))
            self.off += pad
        self.secs[name] = (self.off, rows, cols)
        self.parts.append(b.reshape(-1))
        self.off += rows * cols

    def finish(self):
        pad = (-self.off) % 128
        if pad:
            self.parts.append(np.zeros(pad, np.int16))
            self.off += pad
        return np.concatenate(self.parts)


def host_prep(inputs, cfg):
    """Returns (in_maps, C) -- per-core single-blob inputs."""
    N, E, CORES = cfg["N"], cfg["E"], cfg["CORES"]
    SHARD, BLK, BLOCKS = cfg["SHARD"], cfg["BLK"], cfg["BLOCKS"]
    L, D, H = cfg["L"], cfg["D"], cfg["H"]
    bf16 = _np_bf16()

    x = np.ascontiguousarray(np.asarray(inputs["x"], np.float32))
    ei = np.asarray(inputs["edge_index"], np.int64)
    src = ei[0]
    dst = ei[1]
    loops = np.arange(N, dtype=np.int64)
    src = np.concatenate([src, loops])
    dst = np.concatenate([dst, loops])
    order = np.argsort(dst, kind="stable")
    src, dst = src[order], dst[order]

    nblk_total = N // BLK
    blk_of = dst // BLK
    counts = np.bincount(blk_of, minlength=nblk_total)
    C = int(np.ceil(counts.max() / 128))
    cfg["C"] = C
    slots = C * 128

    srcs = np.zeros((CORES, BLOCKS, slots), np.int64)
    dloc = np.full((CORES, BLOCKS, slots), -1.0, np.float32)
    dsts = np.zeros((CORES, BLOCKS, slots), np.int64)
    starts = np.concatenate([[0], np.cumsum(counts)])
    for b in range(nblk_total):
        core, blk = b // BLOCKS, b % BLOCKS
        s, e = int(starts[b]), int(starts[b + 1])
        n = e - s
        srcs[core, blk, :n] = src[s:e]
        dsts[core, blk, :n] = dst[s:e]
        dsts[core, blk, n:] = b * BLK  # valid row for pad reads
        dloc[core, blk, :n] = (dst[s:e] - b * BLK).astype(np.float32)

    def wrap_idx(a):
        # a: [BLOCKS, ns] int -> int16 [128, BLOCKS * ns//16], slot j of
        # block b at [j % 16 (replicated x8), b*ns//16 + j//16]
        ns = a.shape[1]
        a16 = a.reshape(BLOCKS, ns // 16, 16).transpose(0, 2, 1)
        a16 = a16.reshape(1, BLOCKS * 16, ns // 16)
        cols = np.concatenate(
            [a16[0, b * 16:(b + 1) * 16, :] for b in range(BLOCKS)],
            axis=1)  # [16, BLOCKS*ns//16]
        assert a.max() < 2 ** 15
        return np.tile(cols.astype(np.int16), (8, 1))

    # dloc layout [128, BLOCKS*C]: [p, b*C + ch] = slot ch*128+p of block b
    dloc_t = dloc.reshape(CORES, BLOCKS, C, 128).transpose(0, 3, 1, 2) \
                 .reshape(CORES, 128, BLOCKS * C)

    Wg = np.asarray(inputs["W_gat"], np.float32)     # [L, D, H*D]
    a_s = np.asarray(inputs["att_src"], np.float32)  # [L, H, D]
    a_d = np.asarray(inputs["att_dst"], np.float32)
    wasd = np.zeros((D, L * 2 * H), np.float32)
    for l in range(L):
        for h in range(H):
            Wh = Wg[l][:, h * D:(h + 1) * D]
            wasd[:, l * 2 * H + h] = Wh @ a_s[l, h]
            wasd[:, l * 2 * H + H + h] = Wh @ a_d[l, h]

    def col3(name):  # [L, D] -> [D, L]
        return np.ascontiguousarray(np.asarray(inputs[name], np.float32).T)

    blob = _Blob()
    # 0.25 head-mean folded into W_gat; [D, L*H*D] d-major
    blob.add("wgat", (0.25 * Wg).transpose(1, 0, 2).reshape(D, L * H * D)
             .astype(bf16))
    blob.add("w1", np.asarray(inputs["W1"], np.float32)
             .transpose(1, 0, 2).reshape(D, L * D).astype(bf16))
    blob.add("w2", np.asarray(inputs["W2"], np.float32)
             .transpose(1, 0, 2).reshape(D, L * D).astype(bf16))
    blob.add("wasd", wasd)
    blob.add("bg", col3("bias_gat"))
    blob.add("b1", col3("b1"))
    blob.add("b2", col3("b2"))
    blob.add("n1", np.asarray(inputs["norm1_w"], np.float32).reshape(1, -1))
    blob.add("n2", np.asarray(inputs["norm2_w"], np.float32).reshape(1, -1))
    blob.add("iota", np.tile(np.arange(BLK, dtype=np.float32), (128, 1))
             .astype(bf16))
    blob.add("ident", np.eye(128, dtype=np.float32))
    blob.add("onescol", np.ones((128, 2), bf16))   # col 0 used
    blob.add("onesrow", np.ones((1, 128), bf16))
    blob.add("onesf", np.ones((128, 2), np.float32))  # col 0 used
    # head-selector for denominator broadcast: hsel[k, h*128+m] = (k==h)
    blob.add("hsel", np.eye(H, dtype=np.float32).repeat(128, axis=1)
             .astype(bf16))
    common_len = blob.off
    common_parts = list(blob.parts)
    common_secs = dict(blob.secs)

    in_maps = []
    blob_len = None
    for c in range(CORES):
        bl = _Blob()
        bl.parts = list(common_parts)
        bl.off = common_len
        bl.secs = dict(common_secs)
        bl.add("idxa", wrap_idx(srcs[c]))
        bl.add("idxb", wrap_idx(dsts[c]))
        bl.add("dloc", dloc_t[c].astype(bf16))
        bl.add("xin", x[c * SHARD:(c + 1) * SHARD])
        buf = bl.finish()
        blob_len = len(buf)
        cfg["SECS"] = bl.secs
        in_maps.append({"blob": buf})
    cfg["BLOB_LEN"] = blob_len
    return in_maps, C


def build_program(cfg, debug=False):
    N, CORES = cfg["N"], cfg["CORES"]
    SHARD, BLK, BLOCKS, C = cfg["SHARD"], cfg["BLK"], cfg["BLOCKS"], cfg["C"]
    L, D, H = cfg["L"], cfg["D"], cfg["H"]
    SECS = cfg["SECS"]
    TW = 256          # packed table row width (bf16 elems) = 512 B
    slots = C * 128
    NQ = cfg.get("NQ", 4)

    nc = bacc.Bacc("TRN2", target_bir_lowering=False, debug=debug,
                   num_devices=CORES, num_swdge_queues=NQ,
                   dynamic_dma_scratch_size=16384)

    blob = nc.dram_tensor("blob", [cfg["BLOB_LEN"]], I16,
                          kind="ExternalInput").ap()
    out = nc.dram_tensor("out", [SHARD, D], F32, kind="ExternalOutput").ap()

    def sec(name, dt=F32):
        off, rows, cols = SECS[name]
        v = blob[off:off + rows * cols].rearrange("(r c) -> r c", c=cols)
        return v if dt == I16 else v.bitcast(dt)

    with tile.TileContext(nc) as tc:
        with tc.tile_pool(name="persist", bufs=1) as pp, \
             tc.tile_pool(name="dram", bufs=1, space="DRAM") as dp, \
             tc.tile_pool(name="gath", bufs=3) as gp, \
             tc.tile_pool(name="sc", bufs=3) as scp, \
             tc.tile_pool(name="chunk", bufs=2) as cp, \
             tc.tile_pool(name="post", bufs=2) as pop, \
             tc.tile_pool(name="psA", bufs=1, space="PSUM") as psA, \
             tc.tile_pool(name="psB", bufs=2, space="PSUM") as psB:

            # ---- persistent SBUF ----
            idxa_s = pp.tile([128, BLOCKS * slots // 16], I16)
            idxb_s = pp.tile([128, BLOCKS * slots // 16], I16)
            dloc_s = pp.tile([128, BLOCKS * C], BF16)
            wasd_s = pp.tile([128, L * 2 * H], F32)
            wgat_s = pp.tile([128, L * H * D], BF16)
            w1_s = pp.tile([128, L * D], BF16)
            w2_s = pp.tile([128, L * D], BF16)
            bg_s = pp.tile([128, L], F32)
            b1_s = pp.tile([128, L], F32)
            b2_s = pp.tile([128, L], F32)
            n1_s = pp.tile([1, L * D], F32)
            n2_s = pp.tile([1, L * D], F32)
            iota_s = pp.tile([128, BLK], BF16)
            ident_s = pp.tile([128, 128], F32)
            onescol_s = pp.tile([128, 2], BF16)
            onesrow_s = pp.tile([1, 128], BF16)
            onesf_s = pp.tile([128, 2], F32)
            hsel_s = pp.tile([4, 4 * 128], BF16)
            xT = pp.tile([128, SHARD], F32)
            eps_s = pp.tile([1, 1], F32)

            dma = nc.sync.dma_start
            dma(idxa_s[:], sec("idxa", I16))
            dma(idxb_s[:], sec("idxb", I16))
            dma(dloc_s[:], sec("dloc", BF16))
            dma(wasd_s[:], sec("wasd"))
            dma(wgat_s[:], sec("wgat", BF16))
            dma(w1_s[:], sec("w1", BF16))
            dma(w2_s[:], sec("w2", BF16))
            dma(bg_s[:], sec("bg"))
            dma(b1_s[:], sec("b1"))
            dma(b2_s[:], sec("b2"))
            dma(n1_s[:], sec("n1"))
            dma(n2_s[:], sec("n2"))
            dma(iota_s[:], sec("iota", BF16))
            dma(ident_s[:], sec("ident"))
            dma(onescol_s[:], sec("onescol", BF16))
            dma(onesrow_s[:], sec("onesrow", BF16))
            dma(onesf_s[:], sec("onesf"))
            dma(hsel_s[:], sec("hsel", BF16))
            nc.vector.memset(eps_s[:], EPS)
            xin = sec("xin", F32)

            # ---- DRAM tables for gather + collective ----
            tshard = dp.tile([SHARD, TW], BF16)
            RP = cfg.get("REPS", 1)
            if CORES > 1:
                aspace = "Local" if "coll" in ABLATE else "Shared"
                tfulls = [dp.tile([N, TW], BF16, addr_space=aspace,
                                  tag=f"tfull{i}", name=f"tfull{i}")
                          for i in range(L * RP)]
            else:
                tfulls = [tshard] * (L * RP)

            # ---- init: transpose input shard to feature-major xT ----
            for b in range(BLOCKS):
                xr = gp.tile([BLK, D], F32, tag="xr")
                nc.sync.dma_start(xr[:], xin[b * BLK:(b + 1) * BLK, :])
                ps_t = psB.tile([D, BLK], F32, tag="pb")
                nc.tensor.transpose(ps_t[:], xr[:], ident_s[:BLK, :BLK])
                nc.scalar.copy(xT[:, b * BLK:(b + 1) * BLK], ps_t[:])

            def rmsnorm(z, nw_row, tag, zout=None):
                """z: SBUF [D, BLK] f32 -> z * rsqrt(mean(z^2)+eps) * w.
                rsqrt = exp(-0.5 * ln(ms + eps)): one act table set."""
                zsq = pop.tile([D, BLK], F32, tag=f"zsq{tag}")
                nc.vector.tensor_mul(zsq[:], z[:], z[:])
                ps_ss = psB.tile([1, BLK], F32, tag="pb_ss", bufs=1)
                nc.tensor.matmul(ps_ss[:], onesf_s[:, 0:1], zsq[:],
                                 start=True, stop=True)
                lnm = pop.tile([1, BLK], F32, tag=f"lnm{tag}")
                nc.scalar.activation(lnm[:], ps_ss[:], ACT.Ln,
                                     scale=1.0 / D, bias=eps_s[:])
                rin = pop.tile([1, BLK], F32, tag=f"rin{tag}")
                nc.scalar.activation(rin[:], lnm[:], ACT.Exp, scale=-0.5)
                ps_rb = psB.tile([D, BLK], F32, tag="pb")
                nc.tensor.matmul(ps_rb[:], nw_row, rin[:],
                                 start=True, stop=True)
                zn = zout if zout is not None else pop.tile(
                    [D, BLK], F32, tag=f"zn{tag}")
                nc.vector.tensor_mul(zn if zout is not None else zn[:],
                                     z[:], ps_rb[:])
                return zn

            for rep in range(cfg.get("REPS", 1)):
             for l in range(L):
                # ---- phase A: packed table rows [x bf16 | scores f32] ----
                for b in range(BLOCKS):
                    xb = xT[:, b * BLK:(b + 1) * BLK]
                    ps_a = psB.tile([BLK, 2 * H], F32, tag="pb")
                    nc.tensor.matmul(ps_a[:], xb,
                                     wasd_s[:, l * 2 * H:(l + 1) * 2 * H],
                                     start=True, stop=True)
                    ps_x = psB.tile([BLK, D], F32, tag="pb")
                    nc.tensor.transpose(ps_x[:], xb, ident_s[:])
                    tt = gp.tile([BLK, TW], BF16, tag="tt")
                    nc.scalar.copy(tt[:, 0:D], ps_x[:])
                    tt_f32 = tt[:].bitcast(F32)  # [BLK, TW//2]
                    nc.scalar.copy(tt_f32[:, 64:64 + 2 * H], ps_a[:])
                    nc.vector.memset(tt[:, D + 4 * H:TW], 0.0)
                    nc.sync.dma_start(tshard[b * BLK:(b + 1) * BLK, :], tt[:])

                # ---- phase B: AllGather ----
                if CORES > 1 and "coll" in ABLATE:
                    for s in range(CORES):
                        nc.sync.dma_start(
                            tfulls[rep * L + l][s * SHARD:(s + 1) * SHARD, :],
                            tshard[:])
                elif CORES > 1:
                    nc.gpsimd.collective_compute(
                        "AllGather", AOT.bypass,
                        replica_groups=[list(range(CORES))],
                        ins=[tshard.opt()], outs=[tfulls[rep * L + l].opt()])

                # ---- phase C/D: edge aggregation + block post ----
                def chunk_phase(b):
                    tf = tfulls[rep * L + l]
                    ga = gp.tile([128, C * TW], BF16, tag="ga")
                    gb = gp.tile([128, C * (TW // 2)], BF16, tag="gb")
                    ic0 = b * (slots // 16)
                    ic1 = (b + 1) * (slots // 16)
                    if "noga" in ABLATE:
                        pass
                    elif "ga" not in ABLATE:
                        nc.gpsimd.dma_gather(
                            ga[:].rearrange("p (c e) -> p c e", e=TW),
                            tf[:], idxa_s[:, ic0:ic1],
                            num_idxs=slots, num_idxs_reg=slots,
                            elem_size=TW, queue_num=(2 * b) % NQ,
                            single_packet=False)
                    else:
                        nc.sync.dma_start(
                            ga[:].rearrange("p (c e) -> p c e", e=TW),
                            tf[0:128 * C, :].rearrange(
                                "(c p) e -> p c e", p=128))
                    if "noga" in ABLATE:
                        pass
                    elif "ga" not in ABLATE:
                        nc.gpsimd.dma_gather(
                            gb[:].rearrange("p (c e) -> p c e", e=TW // 2),
                            tf[:, D:TW], idxb_s[:, ic0:ic1],
                            num_idxs=slots, num_idxs_reg=slots,
                            elem_size=TW // 2, elem_step=TW,
                            queue_num=(2 * b + 1) % NQ, single_packet=False)
                    else:
                        nc.sync.dma_start(
                            gb[:].rearrange("p (c e) -> p c e", e=TW // 2),
                            tf[0:128 * C, D:TW].rearrange(
                                "(c p) e -> p c e", p=128))
                    ga_f = ga[:].bitcast(F32).rearrange(
                        "p (c e) -> p c e", e=TW // 2)
                    gb_f = gb[:].bitcast(F32).rearrange(
                        "p (c e) -> p c e", e=TW // 4)

                    q = scp.tile([128, C * H], F32, tag="q")
                    lr = scp.tile([128, C * H], F32, tag="lr")
                    wex = scp.tile([128, C * H], BF16, tag="wex")
                    if "score" not in ABLATE:
                        nc.vector.tensor_add(
                            q[:].rearrange("p (c h) -> p c h", h=H),
                            ga_f[:, :, 64:64 + H], gb_f[:, :, H:2 * H])
                        nc.scalar.activation(lr[:], q[:], ACT.Prelu,
                                             alpha=NEG_SLOPE)
                        nc.scalar.activation(wex[:], lr[:], ACT.Exp)

                    s0 = cp.tile([128, C * BLK], BF16, tag="s0")
                    sh = cp.tile([128, C * H * BLK], BF16, tag="sh")
                    if "sdve" not in ABLATE:
                        nc.vector.tensor_tensor(
                            s0[:].rearrange("p (c n) -> p c n", n=BLK),
                            dloc_s[:, b * C:(b + 1) * C].unsqueeze(2)
                                .broadcast_to([128, C, BLK]),
                            iota_s[:].unsqueeze(1).broadcast_to(
                                [128, C, BLK]),
                            op=AOT.is_equal)
                        nc.vector.tensor_tensor(
                            sh[:].rearrange("p (c h n) -> p c h n",
                                            h=H, n=BLK),
                            s0[:].rearrange("p (c n) -> p c n", n=BLK)
                                .unsqueeze(2).broadcast_to([128, C, H, BLK]),
                            wex[:].rearrange("p (c h) -> p c h", h=H)
                                .unsqueeze(3).broadcast_to([128, C, H, BLK]),
                            op=AOT.mult)

                    ps_all = psA.tile([D, H * BLK], F32, tag="ps_all",
                                      name=f"ps_all_{l}_{b}", bufs=2)
                    ps_den = psA.tile([H, BLK], F32, tag="ps_den",
                                      name=f"ps_den_{l}_{b}", bufs=2)
                    gav = ga[:].rearrange("p (c e) -> p c e", e=TW)
                    for ch in range(C):
                        shc = sh[:, ch * H * BLK:(ch + 1) * H * BLK]
                        if "smm" not in ABLATE:
                            nc.tensor.matmul(ps_den[:],
                                             wex[:, ch * H:(ch + 1) * H],
                                             s0[:, ch * BLK:(ch + 1) * BLK],
                                             start=(ch == 0),
                                             stop=(ch == C - 1))
                            nc.tensor.matmul(ps_all[:], gav[:, ch, 0:D], shc,
                                             start=(ch == 0),
                                             stop=(ch == C - 1))
                    return ps_all, ps_den

                def post_phase(b, ps_all, ps_den):
                    if "post" in ABLATE:
                        return
                    xb = xT[:, b * BLK:(b + 1) * BLK]
                    rden = pop.tile([H, BLK], BF16, tag="rden")
                    with nc.allow_low_precision(reason="1/den fine in bf16"):
                        nc.vector.reciprocal(rden[:], ps_den[:])
                    ps_rb = psB.tile([128, H * BLK], F32, tag="pb")
                    for h in range(H):
                        nc.tensor.matmul(ps_rb[:, h * BLK:(h + 1) * BLK],
                                         hsel_s[:, h * 128:(h + 1) * 128],
                                         rden[:], start=True, stop=True)
                    rb = pop.tile([128, H * BLK], F32, tag="rb")
                    nc.scalar.copy(rb[:], ps_rb[:])
                    yh = pop.tile([128, H * BLK], BF16, tag="yh")
                    nc.vector.tensor_mul(yh[:], ps_all[:], rb[:])
                    ps_att = psB.tile([D, BLK], F32, tag="pb")
                    for h in range(H):
                        nc.tensor.matmul(
                            ps_att[:],
                            wgat_s[:, (l * H + h) * D:(l * H + h + 1) * D],
                            yh[:, h * BLK:(h + 1) * BLK],
                            start=(h == 0), stop=(h == H - 1))

                    z = pop.tile([D, BLK], F32, tag="z")
                    nc.vector.scalar_tensor_tensor(
                        z[:], ps_att[:], bg_s[:, l:l + 1], xb,
                        op0=AOT.add, op1=AOT.add)
                    zn1 = rmsnorm(z, n1_s[0:1, l * D:(l + 1) * D], "a")

                    zn1h = pop.tile([D, BLK], BF16, tag="zn1h")
                    nc.scalar.copy(zn1h[:], zn1[:])
                    ps_f1 = psB.tile([D, BLK], F32, tag="pb")
                    nc.tensor.matmul(ps_f1[:], w1_s[:, l * D:(l + 1) * D],
                                     zn1h[:], start=True, stop=True)
                    f1 = pop.tile([D, BLK], BF16, tag="f1")
                    nc.scalar.activation(f1[:], ps_f1[:], ACT.Relu,
                                         bias=b1_s[:, l:l + 1])
                    ps_f2 = psB.tile([D, BLK], F32, tag="pb")
                    nc.tensor.matmul(ps_f2[:], w2_s[:, l * D:(l + 1) * D],
                                     f1[:], start=True, stop=True)
                    z3 = pop.tile([D, BLK], F32, tag="z3")
                    nc.vector.scalar_tensor_tensor(
                        z3[:], ps_f2[:], b2_s[:, l:l + 1], zn1[:],
                        op0=AOT.add, op1=AOT.add)
                    rmsnorm(z3, n2_s[0:1, l * D:(l + 1) * D], "b", zout=xb)

                pending = None
                for b in range(BLOCKS):
                    handles = chunk_phase(b)
                    if pending is not None:
                        post_phase(pending[0], pending[1], pending[2])
                    pending = (b, *handles)
                post_phase(pending[0], pending[1], pending[2])

            # ---- output: transpose back to node-major ----
            for b in range(BLOCKS):
                ps_o = psB.tile([BLK, D], F32, tag="pb")
                nc.tensor.transpose(ps_o[:], xT[:, b * BLK:(b + 1) * BLK],
                                    ident_s[:])
                ot = gp.tile([BLK, D], F32, tag="ot")
                nc.scalar.copy(ot[:], ps_o[:])
                nc.sync.dma_start(out[b * BLK:(b + 1) * BLK, :], ot[:])

    nc.compile()
    return nc


FULL_CFG = dict(N=20000, E=320000, CORES=8, SHARD=2500, BLK=125, BLOCKS=20,
                C=None, L=3, D=128, H=4, NQ=4)


def kernel_run(inputs, trace=False):
    cfg = dict(FULL_CFG)
    in_maps, C = host_prep(inputs, cfg)
    nc = build_program(cfg)
    res = run_bass_kernel_spmd(nc, in_maps, list(range(cfg["CORES"])),
                               trace=trace)
    out = np.concatenate([r["out"] for r in res.results], axis=0)
    return out, res


def kernel(**inputs):
    out, _ = kernel_run(inputs)
    return out.astype(np.float32)
